# revision 59
# baseline (speedup 1.0000x reference)
"""Spectral pooling (FFT2 -> crop low freqs -> IFFT2) as dense DFT matmuls on TRN2.

Input  x: (32, 256, 64, 64) fp32  -- channels 0:128 real part, 128:256 imag part
Output y: (32, 256, 32, 32) fp32

Math: per complex image X (64x64), Y = A @ X @ A.T with
  A = sqrt(1/(64*32)) * IDFT32 @ Crop @ DFT64   (32x64 complex)
Sharding: batch dim across 8 cores (4 batches/core), no communication.

HBM I/O is bf16 and pre-packed on the host into the exact SBUF layouts so
every DMA moves >=4KB contiguous runs per partition at full bus rate:
  x_dev[b]  [128, 8192] = x[b] with partitions (xc, h), cols (c, w)
  y_dev[b]  [128, 2048] = raw stage-2 results; host unscrambles + upcasts.

PE cost on TRN2 is the number of MOVING-operand columns streamed (bf16
1 col/cycle; LDWEIGHTS overlaps with streaming). Stage 1 keeps the data
chunk stationary and streams the 64-col combined-complex DFT matrix R1 =
[[ArT,AiT],[-AiT,ArT]]: per 2 images one matmul streaming 64 cols ->
psum1[(j,w), (jj,pc,pp)] = P = A X. Stage 2 is flipped vs. the obvious
scheme: D2r/D2i become the STATIONARY operand and the 64-col sb1 data
slices stream, as adjacent accumulate pairs per quad (start/stop pairs
must be adjacent: interleaving 8 opens then 8 closes on one PSUM tile
corrupts the accumulation):
  pair, per quad: out[(j,yc,n2), (jj,n1)]  = D2r^T @ sb1[pc=0 slice]
                  out                      += D2i^T @ sb1[pc=1 slice]
This halves stage-2 streamed cols (2x64/quad vs 2x128/quad) -> total
8192 streamed cols per batch ~= the 23.4us/core HBM-in roofline (ridge).

Input is DMAed as 8 tiles of [128, 4096] (8KB packets, ~330GB/s on the
HWDGE path), all issued up-front on the sync-engine HWDGE queue (sync is
otherwise idle, so the tile list-scheduler keeps the issues prompt; busy
engines like ACT get their DMA issues reordered behind compute, and
>8 concurrent DMA instructions serialize on the 8 DMAHW proc
semaphores). Outputs go per-batch on the same queue; dmats rides the
scalar queue head. PSUM->SBUF copies are split across DVE and ACT.
Compute is emitted per input tile as [s1(gA), s1(gB), s2(gA), s2(gB)]:
s1(gB) hides gA's copy latency and no stage-2 work crosses a tile
boundary, so when the pipeline is input-paced the output copies don't
inherit the next tile's DMA wait.
All tiles live in ONE SBUF pool and ONE PSUM pool (tags keep the buffer
sets separate): collapsing 7 pools to 2 removed enough per-pool
semaphore bookkeeping from the engine streams to pull the last matmul
in by ~2.5us and trim the teardown -- worth ~4us total.
Measured single-shot NEFF exec (NTFF): ~43.6-45.5us vs 58.2us for the
stage-2 data-stationary revision this session started from."""

import math

import numpy as np

from concourse import bass, mybir
from concourse.bass_utils import run_bass_kernel_spmd
from concourse.tile import TileContext

N_CORES = 8
B_FULL, C2, H, W = 32, 256, 64, 64
HP, WP = 32, 32
BPC = B_FULL // N_CORES  # batches per core
NG = 4  # 2-sg groups per batch (each group = 2048 input cols = 32 images)

F32 = mybir.dt.float32
BF16 = mybir.dt.bfloat16


def _split_multi_waits(nc):
    """This walrus build rejects instructions carrying more than one semaphore
    wait. Hoist extra waits onto same-engine NOPs inserted just before the
    instruction (engine queues execute in order, so blocking is equivalent)."""
    n_split = 0
    for f in nc.m.functions:
        for bb in f.blocks:
            insts = bb.instructions
            out = []
            for inst in insts:
                si = inst.sync_info
                waits = list(si.on_wait) if si and si.on_wait else []
                if len(waits) > 1:
                    si.on_wait = waits[-1:]
                    for w in waits[:-1]:
                        nop = mybir.InstNoOp(
                            name=nc.get_next_instruction_name(),
                            ins=[],
                            outs=[],
                            engine=inst.engine,
                            sync_info=mybir.SyncInfo(on_wait=[w], on_update=[]),
                        )
                        out.append(nop)
                        n_split += 1
                out.append(inst)
            if len(out) != len(insts):
                insts[:] = out
    return n_split


def _a_matrix():
    topf = int(math.ceil(H * 0.5 / 2))  # 16
    midf = H // 2 + topf  # 48
    F = np.exp(-2j * np.pi * np.outer(np.arange(H), np.arange(H)) / H)
    G = np.exp(2j * np.pi * np.outer(np.arange(HP), np.arange(HP)) / HP)
    keep = list(range(topf)) + list(range(midf, H))
    S = np.zeros((HP, H))
    S[np.arange(HP), keep] = 1
    return (G @ S @ F) / np.sqrt(H * W * HP * WP) ** 0.5


def _dft_constants():
    """[128, 320] f32: R1 (64 cols) | D2r (128) | D2i (128)."""
    A = _a_matrix()
    ArT = A.real.astype(np.float32).T  # [64, 32]
    AiT = A.imag.astype(np.float32).T

    R1 = np.block([[ArT, AiT], [-AiT, ArT]])  # [128(xc,h), 64(pc,p)]
    C2r = np.concatenate([ArT, AiT], axis=1)  # [64(w), 64(yc,p2)]
    C2i = np.concatenate([-AiT, ArT], axis=1)
    D2r = np.zeros((128, 128), np.float32)
    D2i = np.zeros((128, 128), np.float32)
    D2r[:64, :64] = C2r
    D2r[64:, 64:] = C2r
    D2i[:64, :64] = C2i
    D2i[64:, 64:] = C2i
    return np.concatenate([R1, D2r, D2i], axis=1)


def build_program(reps: int = 1, split_waits: bool = True,
                  loop_n: int | None = None,
                  p1b: int = 4, p2b: int = 3, s1b: int = 6,
                  outb: int = 2, in_eng: str = "sync", in_cols: int = 4096,
                  out_eng: str = "sync", out_split: int = 1,
                  copy_eng: str = "va", lag: str = "tile",
                  gate: bool = False):
    """reps > 1 unrolls the whole pipeline in-NEFF over the same data so the
    marginal cost per rep can be measured without the ~65ms axon dispatch
    overhead. loop_n wraps the unrolled body in a hardware For_i loop."""
    nc = bass.Bass("TRN2", target_bir_lowering=False, debug=False,
                   enable_partition_id=False)
    # cols 0:320 = DFT constant matrices, then BPC batches of 8192 cols:
    # the constants ride in with the first input tile's DMA (one fewer DMA
    # instruction and input tensor; the scalar HWDGE queue drops out)
    xall = nc.dram_tensor("x", [128, 320 + BPC * 8192], BF16,
                          kind="ExternalInput").ap()
    y = nc.dram_tensor("y", [BPC, 128, 2048], BF16, kind="ExternalOutput").ap()

    def xsl(b, off, w):
        return xall[:, 320 + 8192 * b + off : 320 + 8192 * b + off + w]

    with TileContext(nc) as tc:
        with (
            tc.tile_pool(name="sb", bufs=max(s1b, BPC * (8192 // in_cols))
                         ) as sbpool,
            tc.tile_pool(name="ps", bufs=max(p1b, p2b), space="PSUM") as pspool,
        ):
            # one SBUF + one PSUM pool (tags keep buffer sets separate):
            # fewer pools -> fewer teardown drains/sem-clears in the
            # profiled window
            cpool = ipool = ipoolB = s1pool = opool = sbpool
            p1pool = p2pool = pspool
            consts = {}

            def emit_body():
                for _ in range(reps):
                    emit_rep()

            sb_outs = {}

            def emit_rep():
                # prefetch the entire input up-front on the sync HWDGE
                # queue (8KB packets; sync is otherwise idle so the list
                # scheduler keeps the issues prompt). The first tile also
                # carries the 320 constant cols.
                tins = {}
                t0 = ipool.tile([128, 320 + 4096], BF16, tag="tin0",
                                name="tin0")
                nc.sync.dma_start(out=t0, in_=xall[:, 0 : 320 + 4096])
                consts["r1"] = t0[:, 0:64]
                consts["d2r"] = t0[:, 64:192]
                consts["d2i"] = t0[:, 192:320]
                tins[(0, 0)] = t0[:, 320 : 320 + 2048]
                tins[(0, 1)] = t0[:, 320 + 2048 : 320 + 4096]
                for b in range(BPC):
                    for hb in range(2):
                        if b == 0 and hb == 0:
                            continue
                        t = ipool.tile([128, 4096], BF16, tag="tin",
                                       name="tin")
                        nc.sync.dma_start(out=t, in_=xsl(b, 4096 * hb, 4096))
                        for k in range(2):
                            tins[(b, 2 * hb + k)] = t[
                                :, 2048 * k : 2048 * (k + 1)
                            ]
                # per input tile (= 2 groups): s1(gA), s1(gB), s2(gA), s2(gB).
                # s1(gB) hides gA's PSUM->SBUF copy latency, and no stage-2
                # work waits across a tile boundary (which would inherit the
                # next tile's DMA wait when the pipeline is input-starved).
                if lag == "tile":
                    for b in range(BPC):
                        for hb in range(2):
                            if gate and hb == 0 and b < BPC - 1:
                                # gate this tile-pair's burst on the next
                                # tile too: a dummy LDWEIGHTS makes the PE
                                # wait until ~5.2us of work is resident, so
                                # bursts exceed the ~3us p-state ramp and run
                                # at full clock (ungated for the last pair
                                # to keep the tail short)
                                nc.tensor.ldweights(
                                    weights=tins[(b, 2)][:, 0:64]
                                )
                            gA, gB = 2 * hb, 2 * hb + 1
                            sA = emit_s1(tins[(b, gA)])
                            sB = emit_s1(tins[(b, gB)])
                            emit_s2(sA, b, gA)
                            emit_s2(sB, b, gB)
                else:
                    pending = None
                    for b in range(BPC):
                        for g in range(NG):
                            sb1s = emit_s1(tins[(b, g)])
                            if pending is not None:
                                emit_s2(*pending)
                            pending = (sb1s, b, g)
                    emit_s2(*pending)

            def emit_s1(tin):
                # stage 1: per sg (4 quads), data stationary, stream r1
                sb1s = []
                for sh in range(2):
                    ps1 = p1pool.tile([128, 512], F32, tag="ps1")
                    for q4 in range(4):
                        for jj in range(2):
                            lo = 1024 * sh + 256 * q4 + 128 * jj
                            nc.tensor.matmul(
                                out=ps1[:, 128 * q4 + 64 * jj :
                                        128 * q4 + 64 * jj + 64],
                                lhsT=tin[:, lo : lo + 128],
                                rhs=consts["r1"],
                                start=True,
                                stop=True,
                                tile_position=(0, 0),
                            )
                    # deinterleave pc: ps1 cols (q,jj,pc,pp) -> sb1 cols
                    # (q,pc,jj,pp) so stage-2 rhs slices are contiguous.
                    sb1 = s1pool.tile([128, 512], BF16, tag="sb1")
                    ps1v = ps1.rearrange(
                        "z (q jj pc pp) -> z q jj pc pp", q=4, jj=2, pc=2, pp=32
                    )
                    sb1v = sb1.rearrange(
                        "z (q pc jj pp) -> z q pc jj pp", q=4, pc=2, jj=2, pp=32
                    )
                    for pc in range(2):
                        if sh == 0:
                            nc.vector.tensor_copy(
                                out=sb1v[:, :, pc], in_=ps1v[:, :, :, pc]
                            )
                        elif copy_eng == "vg":
                            nc.gpsimd.tensor_copy(
                                out=sb1v[:, :, pc], in_=ps1v[:, :, :, pc]
                            )
                        else:
                            nc.scalar.copy(out=sb1v[:, :, pc], in_=ps1v[:, :, :, pc])
                    sb1s.append(sb1)
                return sb1s

            def emit_s2(sb1s, b, g):
                # stage 2: D2r/D2i stationary, stream sb1 slices (2 phases)
                if g == 0:
                    sb_outs[b] = opool.tile(
                        [128, 2048], BF16, tag="sb_out", name="sb_out"
                    )
                ps2 = p2pool.tile([128, 512], F32, tag="ps2")
                for sh in range(2):
                    for q4 in range(4):
                        for phase, dmat in ((0, consts["d2r"]), (1, consts["d2i"])):
                            nc.tensor.matmul(
                                out=ps2[:, 64 * (4 * sh + q4) :
                                        64 * (4 * sh + q4) + 64],
                                lhsT=dmat,
                                rhs=sb1s[sh][:, 128 * q4 + 64 * phase :
                                             128 * q4 + 64 * phase + 64],
                                start=(phase == 0),
                                stop=(phase == 1),
                                tile_position=(0, 0),
                            )
                o = sb_outs[b][:, 512 * g : 512 * g + 512]
                if g % 2 == 1:
                    nc.vector.tensor_copy(out=o, in_=ps2)
                elif copy_eng == "vg":
                    nc.gpsimd.tensor_copy(out=o, in_=ps2)
                else:
                    nc.scalar.copy(out=o, in_=ps2)
                if out_eng == "mix3s":
                    # y0-y2 ride the scalar HWDGE queue (data-gated anyway),
                    # keeping the sync queue's descriptor slots for input;
                    # only the latency-critical last y goes on idle sync.
                    oeng = nc.sync if b == BPC - 1 else nc.scalar
                else:
                    oeng = {"scalar": nc.scalar, "sync": nc.sync,
                            "gpsimd": nc.gpsimd}[out_eng]
                if out_split == 2:
                    if g == 1:
                        oeng.dma_start(out=y[b][:, 0:1024],
                                       in_=sb_outs[b][:, 0:1024])
                    elif g == NG - 1:
                        oeng.dma_start(out=y[b][:, 1024:2048],
                                       in_=sb_outs[b][:, 1024:2048])
                elif g == NG - 1:
                    oeng.dma_start(out=y[b], in_=sb_outs[b])

            if loop_n is None:
                emit_body()
            else:
                with tc.For_i(0, loop_n):
                    emit_body()
    if split_waits:
        _split_multi_waits(nc)
    return nc


def _bf16(a: np.ndarray) -> np.ndarray:
    return a.astype(mybir.dt.np(BF16))


def _pack_x(x_shard: np.ndarray) -> np.ndarray:
    """[BPC, 256, 64, 64] f32 -> [BPC, 128, 8192] bf16, partitions (xc, h),
    cols (c, w)."""
    b = x_shard.shape[0]
    xr = x_shard.reshape(b, 2, 128, 64, 64).transpose(0, 1, 3, 2, 4)
    return _bf16(np.ascontiguousarray(xr).reshape(b, 128, 8192))


def _unpack_y(y_dev: np.ndarray) -> np.ndarray:
    """[BPC, 128, 2048] bf16 -> [BPC, 256, 32, 32] f32.

    part = 64*j + 32*yc + n2 ; col = 512*g + 64*(4*sh+q4) + 32*jj + n1 ;
    channel = 128*yc + 32*g + 16*sh + 4*q4 + 2*jj + j."""
    b = y_dev.shape[0]
    a = y_dev.astype(np.float32).reshape(b, 2, 2, 32, 4, 2, 4, 2, 32)
    #                                       j yc n2  g sh q4 jj n1
    a = a.transpose(0, 2, 4, 5, 6, 7, 1, 8, 3)  # b yc g sh q4 jj j n1 n2
    return np.ascontiguousarray(a).reshape(b, 256, 32, 32)


def _make_in_map(x_shard: np.ndarray, dmats: np.ndarray) -> dict:
    xp = _pack_x(x_shard)  # [BPC, 128, 8192]
    b = xp.shape[0]
    xflat = np.ascontiguousarray(xp.transpose(1, 0, 2)).reshape(128, b * 8192)
    return {"x": np.concatenate([_bf16(dmats), xflat], axis=1)}


_CACHED = {}


def _get_program():
    if "nc" not in _CACHED:
        _CACHED["nc"] = build_program()
        _CACHED["consts"] = _dft_constants()
    return _CACHED["nc"], _CACHED["consts"]


def kernel(x: np.ndarray) -> np.ndarray:
    assert x.shape == (B_FULL, C2, H, W) and x.dtype == np.float32
    nc, dmats = _get_program()
    in_maps = [
        _make_in_map(x[BPC * k : BPC * (k + 1)], dmats)
        for k in range(N_CORES)
    ]
    res = run_bass_kernel_spmd(nc, in_maps, list(range(N_CORES)))
    out = np.concatenate(
        [_unpack_y(np.asarray(res.results[k]["y"])) for k in range(N_CORES)],
        axis=0,
    )
    return out.astype(np.float32, copy=False)


if __name__ == "__main__":
    rng = np.random.default_rng(0)
    x = rng.standard_normal((B_FULL, C2, H, W)).astype(np.float32)
    y = kernel(x)
    print("kernel output", y.shape, y.dtype)
